# revision 46
# baseline (speedup 1.0000x reference)
"""TRN2 Bass kernel for nn_Attention_369367187796 (fused single-sweep).

Reference (B=4, DX=1024, N=4096, DQ=DK=DV=1024, fp32):
    Q = Wq @ x[b]; K = Wk @ x[b]; V = Wv @ x[b]
    scores = Q @ K.T   (contract n)
    p = softmax(scores / sqrt(DQ), axis=q)     <- softmax over q!
    out[q,n] = sum_k p[q,k] V[k,n]

Sharding: 8 cores = 4 batches x 2 k-halves. Each core: full Q, its half of
K and V, scoresT[k_half, q] (softmax over q = free axis, fully local), and
the partial out[q,n] over its k-half. Host sums the two partials per batch.

Design (sim 520us baseline -> 235us; rel err 9.322923e-3 vs the 2e-2
gate, HW-validated, bit-stable across runs):
  - Gram-matrix scores: scores = K QT = Wk (x xT) Wq'T. G = x xT is
    computed once per core (512 matmuls) replacing BOTH the Q projection
    (512) and K projection (256), and the scores contraction collapses
    from n=4096 to d=1024 (64+72 small matmuls). G accumulates
    chunk-wise in PSUM, flushes into f32r SBUF tiles, and is rounded to
    bf16 for the AT = G Wk and scoresT = AT.T Wq' matmuls (matmuls
    cannot mix f32r with bf16 operands).
  - G is symmetric: only the upper triangle is computed (contiguous
    spans per d-sub row, split at 512 for PSUM; bf16 matmuls run 1
    cyc/row at any free width). All 28 lower [128,128] bf16 tiles are
    filled by DMA-XBAR-transposing the mirror tiles (zero PE cost,
    bit-identical values). Per-core PE busy ~= 212us.
  - x ships twice (xb [d,n] for the V projection, xt [n,d] for G), both
    bf16; weights ship bf16 (no staging copies needed).
  - K mean-restore is a rank-1 term 1_k (x) vq with vq = 0.5 Wq'(x s)
    computed exactly on the host; added via one K=1 matmul per scores
    psum. The Q mean-restore cancels in the softmax entirely.
  - 1/sqrt(DQ) = 2^-5 (exact) folded into the shipped Wq; softmax is
    exp(s - m) with bias = reduce_max(negate=True), exp on the ACT
    queue, emitted per-kt right after that kt's scores psum flush.
  - V projection runs inside the sweep per chunk (its last-chunk matmuls
    cover the final G-flush latency); out-phase matmuls are kt-outer on
    the first n-chunk (start as soon as p_r[0] lands), qt-major after;
    the last chunk's stores stream behind each eviction (ACT queue).

Layouts (per core):
    G[d, d']:     lhsT = xT [n, d-sub] (bf16), rhs = xT [n, d'] (bf16)
    AT[d', k]:    lhsT = G [d, d'-sub] (bf16), rhs = WkT [d, k] (bf16)
    scoresT[k,q]: lhsT = AT [d', k-sub] (bf16), rhs = WqT [d', q] (bf16)
    out[q,n]:     lhsT = pT [k, q] (f32r),      rhs = V [k, n] (f32r)
  scoresT has q on the free axis (softmax axis), k on partitions.
  V stays SBUF-resident f32r (64KB/partition).
"""

import math

import numpy as np

B_FULL, DX_FULL, N_FULL = 4, 1024, 4096
DQ_FULL = DK_FULL = 1024
N_CORES = 8


def _build_core_kernel(DX, N, DQ, DKH, bench=False, bench_reps=0):
    import concourse.bass as bass
    import concourse.mybir as mybir
    import concourse.tile as tile
    from concourse import bacc

    f32 = mybir.dt.float32
    f32r = mybir.dt.float32r
    bf16 = mybir.dt.bfloat16

    P = 128
    DT = DX // P            # d-tiles (projection contraction)
    CW = 512                # n-chunk width
    NCH = N // CW           # n chunks
    NSUB = CW // P          # n-subtiles per chunk
    NT128 = N // P          # global n tiles of 128
    QC = DQ // 512          # q chunks of 512
    KT = DKH // P           # k tiles of 128
    QT128 = DQ // P         # q tiles (out partitions)
    OCW = 512               # out-phase n-chunk width
    scale = 1.0 / math.sqrt(DQ)

    assert DX % P == 0 and N % CW == 0 and DQ % 512 == 0 and DKH % P == 0

    nc = bacc.Bacc(None, target_bir_lowering=False, debug=False)

    kind_big = "Internal" if bench else "ExternalInput"
    kind_out = "Internal" if bench else "ExternalOutput"
    xb = nc.dram_tensor("xb", [DX, N], bf16, kind=kind_big)
    xt = nc.dram_tensor("xt", [N, DX], bf16, kind=kind_big)
    wqt = nc.dram_tensor("wqt", [DX, DQ], bf16, kind=kind_big)
    wkt = nc.dram_tensor("wkt", [DX, DKH], bf16, kind=kind_big)
    wvt = nc.dram_tensor("wvt", [DX, DKH], bf16, kind=kind_big)
    vq = nc.dram_tensor("vq", [1, DQ], f32, kind=kind_big)
    # tiny input consumed into one output element (value 0 at rest): lets a
    # benchmark chain data dependencies between repeated NEFF executions
    seed = nc.dram_tensor("seed", [1, 1], f32, kind="ExternalInput")
    out = nc.dram_tensor("out", [DQ, N], f32, kind=kind_out)
    sink = (nc.dram_tensor("sink", [1, 1], f32, kind="ExternalOutput")
            if bench else None)

    xv = xb.ap().rearrange("(dt p) n -> p dt n", p=P)
    xtv = xt.ap().rearrange("(t p) d -> p t d", p=P)
    wqv = wqt.ap().rearrange("(dt p) q -> p dt q", p=P)
    wkv = wkt.ap().rearrange("(dt p) k -> p dt k", p=P)
    wvv = wvt.ap().rearrange("(dt p) k -> p dt k", p=P)

    with tile.TileContext(nc) as tc:
        with (
            tc.tile_pool(name="ps", bufs=8, space="PSUM") as ps,
            tc.tile_pool(name="pvres", bufs=1) as pvres,
            tc.tile_pool(name="pscore", bufs=1) as pscore,
            tc.tile_pool(name="ppr", bufs=1) as ppr,
            tc.tile_pool(name="psmx", bufs=1) as psmx,
            tc.tile_pool(name="pstat", bufs=2) as pstat,
            tc.tile_pool(name="pwqk", bufs=1) as pwqk,
            tc.tile_pool(name="psh", bufs=1) as psh,
            tc.tile_pool(name="pg", bufs=1) as pg,
        ):
            # V resident f32r (64KB/partition); scoresT f32 accumulators
            v_res = pvres.tile([P, KT, N], f32r, tag="vres", name="v_res")
            scores_sb = [pscore.tile([P, DQ], f32, tag=f"sc{kt}",
                                     name=f"scores{kt}") for kt in range(KT)]
            p_r = [ppr.tile([P, DQ], f32r, tag=f"pr{kt}", name=f"p{kt}")
                   for kt in range(KT)]

            def softmax_kt(kt):
                # 1/sqrt(DQ) is folded into the shipped Wq, so scores are
                # pre-scaled: exp(s - m) with bias = negated row max
                nm = pstat.tile([P, 1], f32, tag="nm", name=f"nm{kt}")
                den = pstat.tile([P, 1], f32, tag="den", name=f"den{kt}")
                rden = pstat.tile([P, 1], f32, tag="rden", name=f"rden{kt}")
                nc.vector.reduce_max(nm[:], scores_sb[kt][:],
                                     axis=mybir.AxisListType.X, negate=True)
                e = psmx.tile([P, DQ], f32, tag="e", name=f"e{kt}")
                nc.scalar.activation(
                    e[:], scores_sb[kt][:],
                    mybir.ActivationFunctionType.Exp,
                    bias=nm[:], accum_out=den[:])
                nc.vector.reciprocal(rden[:], den[:])
                nc.vector.tensor_scalar_mul(p_r[kt][:], e[:], rden[:])

            rep_cm = tc.For_i(0, bench_reps, 1) if bench_reps else None
            if rep_cm is not None:
                rep_cm.__enter__()

            # ------------- fused sweep: Gram matrix G = x xT + V ---------
            # scores = K QT = Wk (x xT) Wq'T: G replaces both the Q and K
            # projections (the big win: 512+256 matmuls -> 512 for G, and
            # the scores contraction shrinks from n=4096 to d=1024)
            with (
                tc.tile_pool(name="pwv", bufs=1) as pwv,
                tc.tile_pool(name="pxc", bufs=2) as pxc,
                tc.tile_pool(name="pxt", bufs=2) as pxt,
            ):
                # G accumulates through f32r tiles [d-sub 128, d' 1024]
                g_sb = [pg.tile([P, DX], f32r, tag=f"g{ds_}",
                                name=f"g{ds_}") for ds_ in range(DT)]

                xcs = [pxc.tile([P, DT, CW], bf16, tag="xc", name=f"xc{c}")
                       for c in range(2)]
                xts = [pxt.tile([P, NSUB, DX], bf16, tag="xt",
                                name=f"xt{c}") for c in range(2)]
                for nt in range(NSUB):
                    nc.sync.dma_start(xts[0][:, bass.ds(nt, 1)],
                                      xtv[:, bass.ds(nt, 1)])
                nc.sync.dma_start(xcs[0][:], xv[:, :, bass.ds(0, CW)])

                wq_sb = [pwqk.tile([P, 1, DQ], bf16, tag=f"wq{dt}",
                                   name=f"wq{dt}") for dt in range(DT)]
                wk_sb = [pwqk.tile([P, 1, DKH], bf16, tag=f"wk{dt}",
                                   name=f"wk{dt}") for dt in range(DT)]
                wv_sb = [pwv.tile([P, 1, DKH], bf16, tag=f"wv{dt}",
                                  name=f"wv{dt}") for dt in range(DT)]
                vq_sb = psh.tile([1, DQ], f32, tag="vq_sb")
                vq_r = psh.tile([1, DQ], f32r, tag="vq_r")
                ones_r = psh.tile([1, P], f32r, tag="ones_r")
                ones_f = psh.tile([1, P], f32, tag="ones_f")
                for dt in range(DT):
                    d1 = bass.ds(dt, 1)
                    nc.scalar.dma_start(wv_sb[dt][:], wvv[:, d1])
                    if dt == 0:
                        nc.scalar.dma_start(vq_sb[:], vq.ap())
                nc.gpsimd.memset(ones_f[:], 1.0)
                nc.vector.tensor_copy(ones_r[:], ones_f[:])
                nc.vector.tensor_copy(vq_r[:], vq_sb[:])
                # chunk-1 prefetches from the ACT queue (no WAR dep to pace
                # them); later ones are paced by the 2-gen buffers
                nc.scalar.dma_start(xts[1][:], xtv[:, bass.ds(NSUB, NSUB)])
                nc.scalar.dma_start(xcs[1][:], xv[:, :, bass.ds(CW, CW)])

                for c in range(NCH):
                    xtc = xts[c % 2]
                    xc = xcs[c % 2]
                    if c == 2:
                        # wq/wk aren't read until the AT/scores phase;
                        # loading them here keeps the startup HBM burst
                        # for the x stream
                        for dt in range(DT):
                            d1 = bass.ds(dt, 1)
                            nc.scalar.dma_start(wk_sb[dt][:], wkv[:, d1])
                            nc.scalar.dma_start(wq_sb[dt][:], wqv[:, d1])
                    if c >= 1 and c + 1 < NCH:
                        nc.sync.dma_start(
                            xts[(c + 1) % 2][:],
                            xtv[:, bass.ds((c + 1) * NSUB, NSUB)])
                        nc.sync.dma_start(
                            xcs[(c + 1) % 2][:],
                            xv[:, :, bass.ds((c + 1) * CW, CW)])

                    # G[d, d'] += sum_n xT[n, d] xT[n, d'] for this chunk
                    # G is symmetric: compute only upper-tri spans
                    for ds_ in range(DT):
                        dsl = bass.ds(ds_ * P, P)
                        pieces = []
                        st = ds_ * P
                        while st < DX:
                            w = min(512, DX - st)
                            pieces.append((st, w))
                            st += w
                        g_ps = {st: ps.tile([P, w], f32, tag="ps",
                                            name=f"gps{c}_{ds_}_{st}")
                                for st, w in pieces}
                        for nt in range(NSUB):
                            for st, w in pieces:
                                nc.tensor.matmul(
                                    g_ps[st][:], xtc[:, nt, dsl],
                                    xtc[:, nt, bass.ds(st, w)],
                                    start=(nt == 0), stop=(nt == NSUB - 1))
                        for st, w in pieces:
                            dcs = bass.ds(st, w)
                            if c == 0:
                                nc.vector.tensor_copy(
                                    g_sb[ds_][:, dcs], g_ps[st][:])
                            else:
                                nc.vector.tensor_add(
                                    g_sb[ds_][:, dcs], g_sb[ds_][:, dcs],
                                    g_ps[st][:])

                    # V projection: psum [v-128, n-chunk]; on the last
                    # chunk it also covers the final G flush latency
                    for vt in range(KT):
                        vps = ps.tile([P, CW], f32, tag="ps",
                                      name=f"vps{c}_{vt}")
                        vsl = bass.ds(vt * P, P)
                        for dt in range(DT):
                            nc.tensor.matmul(
                                vps[:], wv_sb[dt][:, 0, vsl], xc[:, dt],
                                start=(dt == 0), stop=(dt == DT - 1))
                        nc.scalar.activation(
                            v_res[:, vt, bass.ds(c * CW, CW)], vps[:],
                            mybir.ActivationFunctionType.Copy)

            # ---- AT = G . Wk (per d'-sub): [d'-part, k] bf16 ----
            # (matmul cannot mix f32r with bf16 operands, so G rounds to
            # bf16 after accumulation; sim rel err 9.1e-3 vs 8.5e-3 f32r)
            with tc.tile_pool(name="pat", bufs=1) as pat:
                g_bf = [pat.tile([P, DX], bf16, tag=f"gb{dt}",
                                 name=f"gb{dt}") for dt in range(DT)]
                for dt in range(DT):
                    usl = bass.ds(dt * P, DX - dt * P)
                    nc.vector.tensor_copy(g_bf[dt][:, usl],
                                          g_sb[dt][:, usl])
                # fill all lower tiles via DMA XBAR transpose
                for a in range(1, DT):
                    for b in range(a):
                        nc.sync.dma_start(
                            g_bf[a][:, bass.ds(b * P, P)],
                            g_bf[b][:, bass.ds(a * P, P)],
                            transpose=True)
                at_sb = [pat.tile([P, DKH], bf16, tag=f"at{ds_}",
                                  name=f"at{ds_}") for ds_ in range(DT)]
                for ds_ in range(DT):
                    dsl = bass.ds(ds_ * P, P)
                    at_ps = ps.tile([P, DKH], f32, tag="ps",
                                    name=f"atps{ds_}")
                    for dt in range(DT):
                        nc.tensor.matmul(at_ps[:], g_bf[dt][:, dsl],
                                         wk_sb[dt][:, 0],
                                         start=(dt == 0),
                                         stop=(dt == DT - 1))
                    nc.vector.tensor_copy(at_sb[ds_][:], at_ps[:])

                # ---- scoresT[k, q] = AT.T Wq' + 1_k (x) vq ----
                for kt in range(KT):
                    ksl = bass.ds(kt * P, P)
                    for qc in range(QC):
                        qsl = bass.ds(qc * 512, 512)
                        s_ps = ps.tile([P, 512], f32, tag="ps",
                                       name=f"sps{kt}_{qc}")
                        for dt in range(DT):
                            nc.tensor.matmul(
                                s_ps[:], at_sb[dt][:, ksl],
                                wq_sb[dt][:, 0, qsl],
                                start=(dt == 0), stop=False)
                        # K mean-restore: rank-1, host-computed vq
                        nc.tensor.matmul(s_ps[:], ones_r[:],
                                         vq_r[:, qsl],
                                         start=False, stop=True)
                        nc.vector.tensor_copy(scores_sb[kt][:, qsl],
                                              s_ps[:])
                    softmax_kt(kt)

            # ---------------- softmax over q + out = pT.T @ V -------------
            with (
                tc.tile_pool(name="pout", bufs=4) as pout,
                tc.tile_pool(name="pseed", bufs=1) as pseed,
            ):
                seed_sb = pseed.tile([1, 1], f32, tag="seed")
                nc.sync.dma_start(seed_sb[:], seed.ap())
                outv = out.ap().rearrange("(qt p) n -> p qt n", p=P)
                NOC = N // OCW
                for c in range(NOC):
                    ncol = bass.ds(c * OCW, OCW)
                    if c == 0:
                        # kt-outer across all 8 psum banks: kt0 matmuls
                        # start as soon as p_r[0] lands, overlapping the
                        # softmax chain of kt1..3
                        ops = [ps.tile([P, OCW], f32, tag="ps",
                                       name=f"ops{c}_{qt}")
                               for qt in range(QT128)]
                        for kt in range(KT):
                            for qt in range(QT128):
                                nc.tensor.matmul(
                                    ops[qt][:],
                                    p_r[kt][:, bass.ds(qt * P, P)],
                                    v_res[:, kt, ncol],
                                    start=(kt == 0), stop=(kt == KT - 1))
                        for qg in range(QT128 // 4):
                            osb = pout.tile([P, 4, OCW], f32, tag="osb")
                            for qi in range(4):
                                qt = qg * 4 + qi
                                nc.vector.tensor_copy(osb[:, qi],
                                                      ops[qt][:])
                                if qt == 0:
                                    nc.vector.tensor_scalar_add(
                                        osb[0:1, 0, 0:1], ops[0][0:1, 0:1],
                                        seed_sb[:])
                                    if sink is not None:
                                        nc.sync.dma_start(
                                            sink.ap(), osb[0:1, 0, 0:1])
                            nc.gpsimd.dma_start(
                                outv[:, qg * 4:(qg + 1) * 4, ncol], osb[:])
                    else:
                        # qt-major: evictions and stores trail each psum
                        # closely (fine-grained tail on the last chunk)
                        for qg in range(QT128 // 4):
                            osb = pout.tile([P, 4, OCW], f32, tag="osb")
                            for qi in range(4):
                                qt = qg * 4 + qi
                                ops = ps.tile([P, OCW], f32, tag="ps",
                                              name=f"ops{c}_{qt}")
                                for kt in range(KT):
                                    nc.tensor.matmul(
                                        ops[:],
                                        p_r[kt][:, bass.ds(qt * P, P)],
                                        v_res[:, kt, ncol],
                                        start=(kt == 0),
                                        stop=(kt == KT - 1))
                                nc.vector.tensor_copy(osb[:, qi], ops[:])
                                if c == NOC - 1:
                                    # store right behind each eviction,
                                    # alternating queues so the tail is
                                    # one 0.25MB transfer, not a backlog
                                    q = nc.scalar if qt % 2 else nc.gpsimd
                                    q.dma_start(
                                        outv[:, qt:qt + 1, ncol],
                                        osb[:, qi:qi + 1])
                            if c < NOC - 1:
                                nc.gpsimd.dma_start(
                                    outv[:, qg * 4:(qg + 1) * 4, ncol],
                                    osb[:])

            if rep_cm is not None:
                rep_cm.__exit__(None, None, None)

    nc.compile()
    return nc


_CACHE = {}


def _get_nc(DX, N, DQ, DKH):
    key = (DX, N, DQ, DKH)
    if key not in _CACHE:
        _CACHE[key] = _build_core_kernel(DX, N, DQ, DKH)
    return _CACHE[key]


def _bf16(a):
    import ml_dtypes
    return np.ascontiguousarray(a.astype(ml_dtypes.bfloat16))


def _run(x, Wq, Wk, Wv, **spmd_kwargs):
    """Run the SPMD kernel; returns (out, BassKernelResults)."""
    from concourse.bass_utils import run_bass_kernel_spmd

    B, DX, N = x.shape
    DQ = Wq.shape[0]
    DK = Wk.shape[0]
    assert (B, DX, N, DQ, DK) == (B_FULL, DX_FULL, N_FULL, DQ_FULL, DK_FULL)
    DKH = DK // 2
    P = 128

    nc = _get_nc(DX, N, DQ, DKH)

    # Wq/Wk shipped mean-removed (entries - 0.5) in bf16. Q needs no
    # restore (cancels in softmax); K's restore is the host-exact
    # sh = 0.5*colsum(x), laid out [p, ntile].
    # 1/sqrt(DQ) folded into Wq: scores arrive pre-scaled for the softmax
    WqT = _bf16((np.ascontiguousarray(Wq.T, dtype=np.float32)
                 - np.float32(0.5)) * np.float32(1.0 / math.sqrt(DQ_FULL)))
    WkT = _bf16(np.ascontiguousarray(Wk.T, dtype=np.float32)
                - np.float32(0.5))
    WvT = _bf16(np.ascontiguousarray(Wv.T, dtype=np.float32))

    in_maps = []
    for c in range(N_CORES):
        b, h = divmod(c, 2)
        hsl = slice(h * DKH, (h + 1) * DKH)
        xbf = _bf16(x[b])
        # K mean-restore, rank-1 in q: v[q] = 0.5 * (Wq' (x colsum(x)))[q]
        # (constant across k -> added to scoresT as ones (x) vq on-chip)
        s = x[b].astype(np.float32).sum(axis=0)
        v = 0.5 * (WqT.astype(np.float32).T
                   @ (xbf.astype(np.float32) @ s))
        in_maps.append({
            "xb": xbf,
            "xt": _bf16(np.ascontiguousarray(x[b].T)),
            "wqt": WqT,
            "wkt": np.ascontiguousarray(WkT[:, hsl]),
            "wvt": np.ascontiguousarray(WvT[:, hsl]),
            "vq": np.ascontiguousarray(v[None, :].astype(np.float32)),
            "seed": np.zeros((1, 1), np.float32),
        })

    res = run_bass_kernel_spmd(nc, in_maps, core_ids=list(range(N_CORES)),
                               **spmd_kwargs)
    out = np.empty((B, DQ, N), np.float32)
    for b in range(B):
        out[b] = res.results[2 * b]["out"] + res.results[2 * b + 1]["out"]
    return out, res


def kernel(x, Wq, Wk, Wv):
    return _run(x, Wq, Wk, Wv)[0]



# revision 49
# speedup vs baseline: 1.2980x; 1.2980x over previous
"""TRN2 Bass kernel for nn_Attention_369367187796 (fused single-sweep).

Reference (B=4, DX=1024, N=4096, DQ=DK=DV=1024, fp32):
    Q = Wq @ x[b]; K = Wk @ x[b]; V = Wv @ x[b]
    scores = Q @ K.T   (contract n)
    p = softmax(scores / sqrt(DQ), axis=q)     <- softmax over q!
    out[q,n] = sum_k p[q,k] V[k,n]

Sharding: 8 cores = 4 batches x 2 k-halves. Each core: full Q, its half of
K and V, scoresT[k_half, q] (softmax over q = free axis, fully local), and
the partial out[q,n] over its k-half. Host sums the two partials per batch.

Design (sim 520us baseline -> 235us; rel err 9.322923e-3 vs the 2e-2
gate, HW-validated, bit-stable across runs):
  - Gram-matrix scores: scores = K QT = Wk (x xT) Wq'T. G = x xT is
    computed once per core (512 matmuls) replacing BOTH the Q projection
    (512) and K projection (256), and the scores contraction collapses
    from n=4096 to d=1024 (64+72 small matmuls). G accumulates
    chunk-wise in PSUM, flushes into f32r SBUF tiles, and is rounded to
    bf16 for the AT = G Wk and scoresT = AT.T Wq' matmuls (matmuls
    cannot mix f32r with bf16 operands).
  - G is symmetric: only the upper triangle is computed (contiguous
    spans per d-sub row, split at 512 for PSUM; bf16 matmuls run 1
    cyc/row at any free width). All 28 lower [128,128] bf16 tiles are
    filled by DMA-XBAR-transposing the mirror tiles (zero PE cost,
    bit-identical values). Per-core PE busy ~= 212us.
  - x ships twice (xb [d,n] for the V projection, xt [n,d] for G), both
    bf16; weights ship bf16 (no staging copies needed).
  - K mean-restore is a rank-1 term 1_k (x) vq with vq = 0.5 Wq'(x s)
    computed exactly on the host; added via one K=1 matmul per scores
    psum. The Q mean-restore cancels in the softmax entirely.
  - 1/sqrt(DQ) = 2^-5 (exact) folded into the shipped Wq; softmax is
    exp(s - m) with bias = reduce_max(negate=True), exp on the ACT
    queue, emitted per-kt right after that kt's scores psum flush.
  - V projection runs inside the sweep per chunk (its last-chunk matmuls
    cover the final G-flush latency); out-phase matmuls are kt-outer on
    the first n-chunk (start as soon as p_r[0] lands), qt-major after;
    the last chunk's stores stream behind each eviction (ACT queue).

Layouts (per core):
    G[d, d']:     lhsT = xT [n, d-sub] (bf16), rhs = xT [n, d'] (bf16)
    AT[d', k]:    lhsT = G [d, d'-sub] (bf16), rhs = WkT [d, k] (bf16)
    scoresT[k,q]: lhsT = AT [d', k-sub] (bf16), rhs = WqT [d', q] (bf16)
    out[q,n]:     lhsT = pT [k, q] (f32r),      rhs = V [k, n] (f32r)
  scoresT has q on the free axis (softmax axis), k on partitions.
  V stays SBUF-resident f32r (64KB/partition).
"""

import math

import numpy as np

B_FULL, DX_FULL, N_FULL = 4, 1024, 4096
DQ_FULL = DK_FULL = 1024
N_CORES = 8


def _build_core_kernel(DX, N, DQ, DKH, bench=False, bench_reps=0):
    import concourse.bass as bass
    import concourse.mybir as mybir
    import concourse.tile as tile
    from concourse import bacc

    f32 = mybir.dt.float32
    f32r = mybir.dt.float32r
    bf16 = mybir.dt.bfloat16

    P = 128
    DT = DX // P            # d-tiles (projection contraction)
    CW = 512                # n-chunk width
    NCH = N // CW           # n chunks
    NSUB = CW // P          # n-subtiles per chunk
    NT128 = N // P          # global n tiles of 128
    QC = DQ // 512          # q chunks of 512
    KT = DKH // P           # k tiles of 128
    QT128 = DQ // P         # q tiles (out partitions)
    OCW = 512               # out-phase n-chunk width
    scale = 1.0 / math.sqrt(DQ)

    assert DX % P == 0 and N % CW == 0 and DQ % 512 == 0 and DKH % P == 0

    nc = bacc.Bacc(None, target_bir_lowering=False, debug=False)

    kind_big = "Internal" if bench else "ExternalInput"
    kind_out = "Internal" if bench else "ExternalOutput"
    xb = nc.dram_tensor("xb", [DX, N], bf16, kind=kind_big)
    xt = nc.dram_tensor("xt", [N, DX], bf16, kind=kind_big)
    wqt = nc.dram_tensor("wqt", [DX, DQ], bf16, kind=kind_big)
    wkt = nc.dram_tensor("wkt", [DX, DKH], bf16, kind=kind_big)
    wvt = nc.dram_tensor("wvt", [DX, DKH], bf16, kind=kind_big)
    vq = nc.dram_tensor("vq", [1, DQ], f32, kind=kind_big)
    # tiny input consumed into one output element (value 0 at rest): lets a
    # benchmark chain data dependencies between repeated NEFF executions
    seed = nc.dram_tensor("seed", [1, 1], f32, kind="ExternalInput")
    out = nc.dram_tensor("out", [DQ, N], f32, kind=kind_out)
    sink = (nc.dram_tensor("sink", [1, 1], f32, kind="ExternalOutput")
            if bench else None)

    xv = xb.ap().rearrange("(dt p) n -> p dt n", p=P)
    xtv = xt.ap().rearrange("(t p) d -> p t d", p=P)
    wqv = wqt.ap().rearrange("(dt p) q -> p dt q", p=P)
    wkv = wkt.ap().rearrange("(dt p) k -> p dt k", p=P)
    wvv = wvt.ap().rearrange("(dt p) k -> p dt k", p=P)

    with tile.TileContext(nc) as tc:
        with (
            tc.tile_pool(name="ps", bufs=8, space="PSUM") as ps,
            tc.tile_pool(name="pvres", bufs=1) as pvres,
            tc.tile_pool(name="pscore", bufs=1) as pscore,
            tc.tile_pool(name="ppr", bufs=1) as ppr,
            tc.tile_pool(name="psmx", bufs=1) as psmx,
            tc.tile_pool(name="pstat", bufs=2) as pstat,
            tc.tile_pool(name="pwqk", bufs=1) as pwqk,
            tc.tile_pool(name="psh", bufs=1) as psh,
            tc.tile_pool(name="pg", bufs=1) as pg,
        ):
            # V resident f32r (64KB/partition); scoresT f32 accumulators
            v_res = pvres.tile([P, KT, N], f32r, tag="vres", name="v_res")
            scores_sb = [pscore.tile([P, DQ], f32, tag=f"sc{kt}",
                                     name=f"scores{kt}") for kt in range(KT)]
            p_r = [ppr.tile([P, DQ], f32r, tag=f"pr{kt}", name=f"p{kt}")
                   for kt in range(KT)]

            def softmax_kt(kt):
                # 1/sqrt(DQ) is folded into the shipped Wq, so scores are
                # pre-scaled: exp(s - m) with bias = negated row max
                nm = pstat.tile([P, 1], f32, tag="nm", name=f"nm{kt}")
                den = pstat.tile([P, 1], f32, tag="den", name=f"den{kt}")
                rden = pstat.tile([P, 1], f32, tag="rden", name=f"rden{kt}")
                nc.vector.reduce_max(nm[:], scores_sb[kt][:],
                                     axis=mybir.AxisListType.X, negate=True)
                e = psmx.tile([P, DQ], f32, tag="e", name=f"e{kt}")
                nc.scalar.activation(
                    e[:], scores_sb[kt][:],
                    mybir.ActivationFunctionType.Exp,
                    bias=nm[:], accum_out=den[:])
                nc.vector.reciprocal(rden[:], den[:])
                nc.vector.tensor_scalar_mul(p_r[kt][:], e[:], rden[:])

            rep_cm = tc.For_i(0, bench_reps, 1) if bench_reps else None
            if rep_cm is not None:
                rep_cm.__enter__()

            # ------------- fused sweep: Gram matrix G = x xT + V ---------
            # scores = K QT = Wk (x xT) Wq'T: G replaces both the Q and K
            # projections (the big win: 512+256 matmuls -> 512 for G, and
            # the scores contraction shrinks from n=4096 to d=1024)
            with (
                tc.tile_pool(name="pwv", bufs=1) as pwv,
                tc.tile_pool(name="pxc", bufs=2) as pxc,
                tc.tile_pool(name="pxt", bufs=2) as pxt,
            ):
                # G accumulates through f32r tiles [d-sub 128, d' 1024]
                g_sb = [pg.tile([P, DX], f32r, tag=f"g{ds_}",
                                name=f"g{ds_}") for ds_ in range(DT)]

                xcs = [pxc.tile([P, DT, CW], bf16, tag="xc", name=f"xc{c}")
                       for c in range(2)]
                xts = [pxt.tile([P, NSUB, DX], bf16, tag="xt",
                                name=f"xt{c}") for c in range(2)]
                for nt in range(NSUB):
                    nc.sync.dma_start(xts[0][:, bass.ds(nt, 1)],
                                      xtv[:, bass.ds(nt, 1)])
                nc.sync.dma_start(xcs[0][:], xv[:, :, bass.ds(0, CW)])

                wq_sb = [pwqk.tile([P, 1, DQ], bf16, tag=f"wq{dt}",
                                   name=f"wq{dt}") for dt in range(DT)]
                wk_sb = [pwqk.tile([P, 1, DKH], bf16, tag=f"wk{dt}",
                                   name=f"wk{dt}") for dt in range(DT)]
                wv_sb = [pwv.tile([P, 1, DKH], bf16, tag=f"wv{dt}",
                                  name=f"wv{dt}") for dt in range(DT)]
                vq_sb = psh.tile([1, DQ], f32, tag="vq_sb")
                vq_r = psh.tile([1, DQ], f32r, tag="vq_r")
                ones_r = psh.tile([1, P], f32r, tag="ones_r")
                ones_f = psh.tile([1, P], f32, tag="ones_f")
                for dt in range(DT):
                    d1 = bass.ds(dt, 1)
                    nc.scalar.dma_start(wv_sb[dt][:], wvv[:, d1])
                    if dt == 0:
                        nc.scalar.dma_start(vq_sb[:], vq.ap())
                nc.gpsimd.memset(ones_f[:], 1.0)
                nc.vector.tensor_copy(ones_r[:], ones_f[:])
                nc.vector.tensor_copy(vq_r[:], vq_sb[:])
                # chunk-1 prefetches from the ACT queue (no WAR dep to pace
                # them); later ones are paced by the 2-gen buffers
                nc.scalar.dma_start(xts[1][:], xtv[:, bass.ds(NSUB, NSUB)])
                nc.scalar.dma_start(xcs[1][:], xv[:, :, bass.ds(CW, CW)])

                for c in range(NCH):
                    xtc = xts[c % 2]
                    xc = xcs[c % 2]
                    if c == 2:
                        # wq/wk aren't read until the AT/scores phase;
                        # loading them here keeps the startup HBM burst
                        # for the x stream
                        for dt in range(DT):
                            d1 = bass.ds(dt, 1)
                            nc.scalar.dma_start(wk_sb[dt][:], wkv[:, d1])
                            nc.scalar.dma_start(wq_sb[dt][:], wqv[:, d1])
                    if c >= 1 and c + 1 < NCH:
                        nc.sync.dma_start(
                            xts[(c + 1) % 2][:],
                            xtv[:, bass.ds((c + 1) * NSUB, NSUB)])
                        nc.sync.dma_start(
                            xcs[(c + 1) % 2][:],
                            xv[:, :, bass.ds((c + 1) * CW, CW)])

                    # G[d, d'] += sum_n xT[n, d] xT[n, d'] for this chunk
                    # G is symmetric: compute only upper-tri spans
                    for ds_ in range(DT):
                        dsl = bass.ds(ds_ * P, P)
                        pieces = []
                        st = ds_ * P
                        while st < DX:
                            w = min(512, DX - st)
                            pieces.append((st, w))
                            st += w
                        g_ps = {st: ps.tile([P, w], f32, tag="ps",
                                            name=f"gps{c}_{ds_}_{st}")
                                for st, w in pieces}
                        for nt in range(NSUB):
                            for st, w in pieces:
                                nc.tensor.matmul(
                                    g_ps[st][:], xtc[:, nt, dsl],
                                    xtc[:, nt, bass.ds(st, w)],
                                    start=(nt == 0), stop=(nt == NSUB - 1))
                        for st, w in pieces:
                            dcs = bass.ds(st, w)
                            if c == 0:
                                nc.vector.tensor_copy(
                                    g_sb[ds_][:, dcs], g_ps[st][:])
                            else:
                                nc.vector.tensor_add(
                                    g_sb[ds_][:, dcs], g_sb[ds_][:, dcs],
                                    g_ps[st][:])

                    # V projection: psum [v-128, n-chunk]; on the last
                    # chunk it also covers the final G flush latency
                    for vt in range(KT):
                        vps = ps.tile([P, CW], f32, tag="ps",
                                      name=f"vps{c}_{vt}")
                        vsl = bass.ds(vt * P, P)
                        for dt in range(DT):
                            nc.tensor.matmul(
                                vps[:], wv_sb[dt][:, 0, vsl], xc[:, dt],
                                start=(dt == 0), stop=(dt == DT - 1))
                        nc.scalar.activation(
                            v_res[:, vt, bass.ds(c * CW, CW)], vps[:],
                            mybir.ActivationFunctionType.Copy)

            # ---- AT = G . Wk (per d'-sub): [d'-part, k] bf16 ----
            # (matmul cannot mix f32r with bf16 operands, so G rounds to
            # bf16 after accumulation; sim rel err 9.1e-3 vs 8.5e-3 f32r)
            with tc.tile_pool(name="pat", bufs=1) as pat:
                g_bf = [pat.tile([P, DX], bf16, tag=f"gb{dt}",
                                 name=f"gb{dt}") for dt in range(DT)]
                for dt in range(DT):
                    usl = bass.ds(dt * P, DX - dt * P)
                    nc.vector.tensor_copy(g_bf[dt][:, usl],
                                          g_sb[dt][:, usl])
                # fill all lower tiles via DMA XBAR transpose,
                # split across both HWDGE rings for 2x throughput
                ti = 0
                tq = [nc.sync, nc.scalar]
                for a in range(1, DT):
                    for b in range(a):
                        tq[ti % 2].dma_start(
                            g_bf[a][:, bass.ds(b * P, P)],
                            g_bf[b][:, bass.ds(a * P, P)],
                            transpose=True)
                        ti += 1
                at_sb = [pat.tile([P, DKH], bf16, tag=f"at{ds_}",
                                  name=f"at{ds_}") for ds_ in range(DT)]
                for ds_ in range(DT):
                    dsl = bass.ds(ds_ * P, P)
                    at_ps = ps.tile([P, DKH], f32, tag="ps",
                                    name=f"atps{ds_}")
                    for dt in range(DT):
                        nc.tensor.matmul(at_ps[:], g_bf[dt][:, dsl],
                                         wk_sb[dt][:, 0],
                                         start=(dt == 0),
                                         stop=(dt == DT - 1))
                    nc.vector.tensor_copy(at_sb[ds_][:], at_ps[:])

                # ---- scoresT[k, q] = AT.T Wq' + 1_k (x) vq ----
                for kt in range(KT):
                    ksl = bass.ds(kt * P, P)
                    for qc in range(QC):
                        qsl = bass.ds(qc * 512, 512)
                        s_ps = ps.tile([P, 512], f32, tag="ps",
                                       name=f"sps{kt}_{qc}")
                        for dt in range(DT):
                            nc.tensor.matmul(
                                s_ps[:], at_sb[dt][:, ksl],
                                wq_sb[dt][:, 0, qsl],
                                start=(dt == 0), stop=False)
                        # K mean-restore: rank-1, host-computed vq
                        nc.tensor.matmul(s_ps[:], ones_r[:],
                                         vq_r[:, qsl],
                                         start=False, stop=True)
                        nc.vector.tensor_copy(scores_sb[kt][:, qsl],
                                              s_ps[:])
                    softmax_kt(kt)

            # ---------------- softmax over q + out = pT.T @ V -------------
            with (
                tc.tile_pool(name="pout", bufs=4) as pout,
                tc.tile_pool(name="pseed", bufs=1) as pseed,
            ):
                seed_sb = pseed.tile([1, 1], f32, tag="seed")
                nc.sync.dma_start(seed_sb[:], seed.ap())
                outv = out.ap().rearrange("(qt p) n -> p qt n", p=P)
                NOC = N // OCW
                for c in range(NOC):
                    ncol = bass.ds(c * OCW, OCW)
                    if c == 0:
                        # kt-outer across all 8 psum banks: kt0 matmuls
                        # start as soon as p_r[0] lands, overlapping the
                        # softmax chain of kt1..3
                        ops = [ps.tile([P, OCW], f32, tag="ps",
                                       name=f"ops{c}_{qt}")
                               for qt in range(QT128)]
                        for kt in range(KT):
                            for qt in range(QT128):
                                nc.tensor.matmul(
                                    ops[qt][:],
                                    p_r[kt][:, bass.ds(qt * P, P)],
                                    v_res[:, kt, ncol],
                                    start=(kt == 0), stop=(kt == KT - 1))
                        for qg in range(QT128 // 4):
                            osb = pout.tile([P, 4, OCW], f32, tag="osb")
                            for qi in range(4):
                                qt = qg * 4 + qi
                                nc.vector.tensor_copy(osb[:, qi],
                                                      ops[qt][:])
                                if qt == 0:
                                    nc.vector.tensor_scalar_add(
                                        osb[0:1, 0, 0:1], ops[0][0:1, 0:1],
                                        seed_sb[:])
                                    if sink is not None:
                                        nc.sync.dma_start(
                                            sink.ap(), osb[0:1, 0, 0:1])
                            q = nc.gpsimd if qg % 2 else nc.sync
                            q.dma_start(
                                outv[:, qg * 4:(qg + 1) * 4, ncol], osb[:])
                    else:
                        # qt-major: evictions and stores trail each psum
                        # closely (fine-grained tail on the last chunk)
                        for qg in range(QT128 // 4):
                            osb = pout.tile([P, 4, OCW], f32, tag="osb")
                            for qi in range(4):
                                qt = qg * 4 + qi
                                ops = ps.tile([P, OCW], f32, tag="ps",
                                              name=f"ops{c}_{qt}")
                                for kt in range(KT):
                                    nc.tensor.matmul(
                                        ops[:],
                                        p_r[kt][:, bass.ds(qt * P, P)],
                                        v_res[:, kt, ncol],
                                        start=(kt == 0),
                                        stop=(kt == KT - 1))
                                if c == NOC - 1 and qt % 2:
                                    # alternate eviction engine on the
                                    # final chunk so the store tail is
                                    # paced by two engines, not one
                                    nc.scalar.activation(
                                        osb[:, qi], ops[:],
                                        mybir.ActivationFunctionType.Copy)
                                else:
                                    nc.vector.tensor_copy(osb[:, qi],
                                                          ops[:])
                                if c == NOC - 1:
                                    # store right behind each eviction,
                                    # alternating queues so the tail is
                                    # one 0.25MB transfer, not a backlog
                                    q = nc.scalar if qt % 2 else nc.gpsimd
                                    q.dma_start(
                                        outv[:, qt:qt + 1, ncol],
                                        osb[:, qi:qi + 1])
                            if c < NOC - 1:
                                # slab stores alternate rings: 16MB of
                                # output needs both to keep pace
                                q = nc.gpsimd if (c + qg) % 2 else nc.sync
                                q.dma_start(
                                    outv[:, qg * 4:(qg + 1) * 4, ncol],
                                    osb[:])

            if rep_cm is not None:
                rep_cm.__exit__(None, None, None)

    nc.compile()
    return nc


_CACHE = {}


def _get_nc(DX, N, DQ, DKH):
    key = (DX, N, DQ, DKH)
    if key not in _CACHE:
        _CACHE[key] = _build_core_kernel(DX, N, DQ, DKH)
    return _CACHE[key]


def _bf16(a):
    import ml_dtypes
    return np.ascontiguousarray(a.astype(ml_dtypes.bfloat16))


def _run(x, Wq, Wk, Wv, **spmd_kwargs):
    """Run the SPMD kernel; returns (out, BassKernelResults)."""
    from concourse.bass_utils import run_bass_kernel_spmd

    B, DX, N = x.shape
    DQ = Wq.shape[0]
    DK = Wk.shape[0]
    assert (B, DX, N, DQ, DK) == (B_FULL, DX_FULL, N_FULL, DQ_FULL, DK_FULL)
    DKH = DK // 2
    P = 128

    nc = _get_nc(DX, N, DQ, DKH)

    # Wq/Wk shipped mean-removed (entries - 0.5) in bf16. Q needs no
    # restore (cancels in softmax); K's restore is the host-exact
    # sh = 0.5*colsum(x), laid out [p, ntile].
    # 1/sqrt(DQ) folded into Wq: scores arrive pre-scaled for the softmax
    WqT = _bf16((np.ascontiguousarray(Wq.T, dtype=np.float32)
                 - np.float32(0.5)) * np.float32(1.0 / math.sqrt(DQ_FULL)))
    WkT = _bf16(np.ascontiguousarray(Wk.T, dtype=np.float32)
                - np.float32(0.5))
    WvT = _bf16(np.ascontiguousarray(Wv.T, dtype=np.float32))

    in_maps = []
    for c in range(N_CORES):
        b, h = divmod(c, 2)
        hsl = slice(h * DKH, (h + 1) * DKH)
        xbf = _bf16(x[b])
        # K mean-restore, rank-1 in q: v[q] = 0.5 * (Wq' (x colsum(x)))[q]
        # (constant across k -> added to scoresT as ones (x) vq on-chip)
        s = x[b].astype(np.float32).sum(axis=0)
        v = 0.5 * (WqT.astype(np.float32).T
                   @ (xbf.astype(np.float32) @ s))
        in_maps.append({
            "xb": xbf,
            "xt": _bf16(np.ascontiguousarray(x[b].T)),
            "wqt": WqT,
            "wkt": np.ascontiguousarray(WkT[:, hsl]),
            "wvt": np.ascontiguousarray(WvT[:, hsl]),
            "vq": np.ascontiguousarray(v[None, :].astype(np.float32)),
            "seed": np.zeros((1, 1), np.float32),
        })

    res = run_bass_kernel_spmd(nc, in_maps, core_ids=list(range(N_CORES)),
                               **spmd_kwargs)
    out = np.empty((B, DQ, N), np.float32)
    for b in range(B):
        out[b] = res.results[2 * b]["out"] + res.results[2 * b + 1]["out"]
    return out, res


def kernel(x, Wq, Wk, Wv):
    return _run(x, Wq, Wk, Wv)[0]



# revision 53
# speedup vs baseline: 1.6099x; 1.2403x over previous
"""TRN2 Bass kernel for nn_Attention_369367187796 (fused single-sweep).

Reference (B=4, DX=1024, N=4096, DQ=DK=DV=1024, fp32):
    Q = Wq @ x[b]; K = Wk @ x[b]; V = Wv @ x[b]
    scores = Q @ K.T   (contract n)
    p = softmax(scores / sqrt(DQ), axis=q)     <- softmax over q!
    out[q,n] = sum_k p[q,k] V[k,n]

Sharding: 8 cores = 4 batches x 2 k-halves. Each core: full Q, its half of
K and V, scoresT[k_half, q] (softmax over q = free axis, fully local), and
the partial out[q,n] over its k-half. Host sums the two partials per batch.

Design (sim 520us baseline -> 235us; rel err 9.322923e-3 vs the 2e-2
gate, HW-validated, bit-stable across runs):
  - Gram-matrix scores: scores = K QT = Wk (x xT) Wq'T. G = x xT is
    computed once per core (512 matmuls) replacing BOTH the Q projection
    (512) and K projection (256), and the scores contraction collapses
    from n=4096 to d=1024 (64+72 small matmuls). G accumulates
    chunk-wise in PSUM, flushes into f32r SBUF tiles, and is rounded to
    bf16 for the AT = G Wk and scoresT = AT.T Wq' matmuls (matmuls
    cannot mix f32r with bf16 operands).
  - G is symmetric: only the upper triangle is computed (contiguous
    spans per d-sub row, split at 512 for PSUM; bf16 matmuls run 1
    cyc/row at any free width). All 28 lower [128,128] bf16 tiles are
    filled by DMA-XBAR-transposing the mirror tiles (zero PE cost,
    bit-identical values). Per-core PE busy ~= 212us.
  - x ships twice (xb [d,n] for the V projection, xt [n,d] for G), both
    bf16; weights ship bf16 (no staging copies needed).
  - K mean-restore is a rank-1 term 1_k (x) vq with vq = 0.5 Wq'(x s)
    computed exactly on the host; added via one K=1 matmul per scores
    psum. The Q mean-restore cancels in the softmax entirely.
  - 1/sqrt(DQ) = 2^-5 (exact) folded into the shipped Wq; softmax is
    exp(s - m) with bias = reduce_max(negate=True), exp on the ACT
    queue, emitted per-kt right after that kt's scores psum flush.
  - V projection runs inside the sweep per chunk (its last-chunk matmuls
    cover the final G-flush latency); out-phase matmuls are kt-outer on
    the first n-chunk (start as soon as p_r[0] lands), qt-major after;
    the last chunk's stores stream behind each eviction (ACT queue).

Layouts (per core):
    G[d, d']:     lhsT = xT [n, d-sub] (bf16), rhs = xT [n, d'] (bf16)
    AT[d', k]:    lhsT = G [d, d'-sub] (bf16), rhs = WkT [d, k] (bf16)
    scoresT[k,q]: lhsT = AT [d', k-sub] (bf16), rhs = WqT [d', q] (bf16)
    out[q,n]:     lhsT = pT [k, q] (f32r),      rhs = V [k, n] (f32r)
  scoresT has q on the free axis (softmax axis), k on partitions.
  V stays SBUF-resident f32r (64KB/partition).
"""

import math

import numpy as np

B_FULL, DX_FULL, N_FULL = 4, 1024, 4096
DQ_FULL = DK_FULL = 1024
N_CORES = 8


def _build_core_kernel(DX, N, DQ, DKH, bench=False, bench_reps=0):
    import concourse.bass as bass
    import concourse.mybir as mybir
    import concourse.tile as tile
    from concourse import bacc

    f32 = mybir.dt.float32
    f32r = mybir.dt.float32r
    bf16 = mybir.dt.bfloat16

    P = 128
    DT = DX // P            # d-tiles (projection contraction)
    CW = 512                # n-chunk width
    NCH = N // CW           # n chunks
    NSUB = CW // P          # n-subtiles per chunk
    NT128 = N // P          # global n tiles of 128
    QC = DQ // 512          # q chunks of 512
    KT = DKH // P           # k tiles of 128
    QT128 = DQ // P         # q tiles (out partitions)
    OCW = 512               # out-phase n-chunk width
    scale = 1.0 / math.sqrt(DQ)

    assert DX % P == 0 and N % CW == 0 and DQ % 512 == 0 and DKH % P == 0

    nc = bacc.Bacc(None, target_bir_lowering=False, debug=False)

    kind_big = "Internal" if bench else "ExternalInput"
    kind_out = "Internal" if bench else "ExternalOutput"
    xb = nc.dram_tensor("xb", [DX, N], bf16, kind=kind_big)
    xt = nc.dram_tensor("xt", [N, DX], bf16, kind=kind_big)
    wqt = nc.dram_tensor("wqt", [DX, DQ], bf16, kind=kind_big)
    wkt = nc.dram_tensor("wkt", [DX, DKH], bf16, kind=kind_big)
    wvt = nc.dram_tensor("wvt", [DX, DKH], bf16, kind=kind_big)
    vq = nc.dram_tensor("vq", [1, DQ], f32, kind=kind_big)
    # tiny input consumed into one output element (value 0 at rest): lets a
    # benchmark chain data dependencies between repeated NEFF executions
    seed = nc.dram_tensor("seed", [1, 1], f32, kind="ExternalInput")
    out = nc.dram_tensor("out", [DQ, N], f32, kind=kind_out)
    sink = (nc.dram_tensor("sink", [1, 1], f32, kind="ExternalOutput")
            if bench else None)

    xv = xb.ap().rearrange("(dt p) n -> p dt n", p=P)
    xtv = xt.ap().rearrange("(t p) d -> p t d", p=P)
    wqv = wqt.ap().rearrange("(dt p) q -> p dt q", p=P)
    wkv = wkt.ap().rearrange("(dt p) k -> p dt k", p=P)
    wvv = wvt.ap().rearrange("(dt p) k -> p dt k", p=P)

    with tile.TileContext(nc) as tc:
        with (
            tc.tile_pool(name="ps", bufs=8, space="PSUM") as ps,
            tc.tile_pool(name="pvres", bufs=1) as pvres,
            tc.tile_pool(name="pscore", bufs=1) as pscore,
            tc.tile_pool(name="ppr", bufs=1) as ppr,
            tc.tile_pool(name="psmx", bufs=1) as psmx,
            tc.tile_pool(name="pstat", bufs=2) as pstat,
            tc.tile_pool(name="pwqk", bufs=1) as pwqk,
            tc.tile_pool(name="psh", bufs=1) as psh,
            tc.tile_pool(name="pg", bufs=1) as pg,
        ):
            # V resident f32r (64KB/partition); scoresT f32 accumulators
            v_res = pvres.tile([P, KT, N], f32r, tag="vres", name="v_res")
            scores_sb = [pscore.tile([P, DQ], f32, tag=f"sc{kt}",
                                     name=f"scores{kt}") for kt in range(KT)]
            p_r = [ppr.tile([P, DQ], f32r, tag=f"pr{kt}", name=f"p{kt}")
                   for kt in range(KT)]

            def softmax_kt(kt):
                # 1/sqrt(DQ) is folded into the shipped Wq, so scores are
                # pre-scaled: exp(s - m) with bias = negated row max
                nm = pstat.tile([P, 1], f32, tag="nm", name=f"nm{kt}")
                den = pstat.tile([P, 1], f32, tag="den", name=f"den{kt}")
                rden = pstat.tile([P, 1], f32, tag="rden", name=f"rden{kt}")
                nc.vector.reduce_max(nm[:], scores_sb[kt][:],
                                     axis=mybir.AxisListType.X, negate=True)
                e = psmx.tile([P, DQ], f32, tag="e", name=f"e{kt}")
                nc.scalar.activation(
                    e[:], scores_sb[kt][:],
                    mybir.ActivationFunctionType.Exp,
                    bias=nm[:], accum_out=den[:])
                nc.vector.reciprocal(rden[:], den[:])
                nc.vector.tensor_scalar_mul(p_r[kt][:], e[:], rden[:])

            rep_cm = tc.For_i(0, bench_reps, 1) if bench_reps else None
            if rep_cm is not None:
                rep_cm.__enter__()

            # ------------- fused sweep: Gram matrix G = x xT + V ---------
            # scores = K QT = Wk (x xT) Wq'T: G replaces both the Q and K
            # projections (the big win: 512+256 matmuls -> 512 for G, and
            # the scores contraction shrinks from n=4096 to d=1024)
            with (
                tc.tile_pool(name="pwv", bufs=1) as pwv,
                tc.tile_pool(name="pxc", bufs=2) as pxc,
                tc.tile_pool(name="pxt", bufs=2) as pxt,
            ):
                # G accumulates through f32r tiles [d-sub 128, d' 1024]
                g_sb = [pg.tile([P, DX], f32r, tag=f"g{ds_}",
                                name=f"g{ds_}") for ds_ in range(DT)]

                xcs = [pxc.tile([P, DT, CW], bf16, tag="xc", name=f"xc{c}")
                       for c in range(2)]
                xts = [pxt.tile([P, NSUB, DX], bf16, tag="xt",
                                name=f"xt{c}") for c in range(2)]
                for nt in range(NSUB):
                    nc.sync.dma_start(xts[0][:, bass.ds(nt, 1)],
                                      xtv[:, bass.ds(nt, 1)])
                nc.sync.dma_start(xcs[0][:], xv[:, :, bass.ds(0, CW)])

                wq_sb = [pwqk.tile([P, 1, DQ], bf16, tag=f"wq{dt}",
                                   name=f"wq{dt}") for dt in range(DT)]
                wk_sb = [pwqk.tile([P, 1, DKH], bf16, tag=f"wk{dt}",
                                   name=f"wk{dt}") for dt in range(DT)]
                wv_sb = [pwv.tile([P, 1, DKH], bf16, tag=f"wv{dt}",
                                  name=f"wv{dt}") for dt in range(DT)]
                vq_sb = psh.tile([1, DQ], f32, tag="vq_sb")
                vq_r = psh.tile([1, DQ], f32r, tag="vq_r")
                ones_r = psh.tile([1, P], f32r, tag="ones_r")
                ones_f = psh.tile([1, P], f32, tag="ones_f")
                for dt in range(DT):
                    d1 = bass.ds(dt, 1)
                    nc.scalar.dma_start(wv_sb[dt][:], wvv[:, d1])
                    if dt == 0:
                        nc.scalar.dma_start(vq_sb[:], vq.ap())
                nc.gpsimd.memset(ones_f[:], 1.0)
                nc.vector.tensor_copy(ones_r[:], ones_f[:])
                nc.vector.tensor_copy(vq_r[:], vq_sb[:])
                # chunk-1 prefetches from the ACT queue (no WAR dep to pace
                # them); later ones are paced by the 2-gen buffers
                nc.scalar.dma_start(xts[1][:], xtv[:, bass.ds(NSUB, NSUB)])
                nc.scalar.dma_start(xcs[1][:], xv[:, :, bass.ds(CW, CW)])

                for c in range(NCH):
                    xtc = xts[c % 2]
                    xc = xcs[c % 2]
                    if c == 2:
                        # wq/wk aren't read until the AT/scores phase;
                        # loading them here keeps the startup HBM burst
                        # for the x stream
                        for dt in range(DT):
                            d1 = bass.ds(dt, 1)
                            nc.scalar.dma_start(wk_sb[dt][:], wkv[:, d1])
                            nc.scalar.dma_start(wq_sb[dt][:], wqv[:, d1])
                    if c >= 1 and c + 1 < NCH:
                        nc.sync.dma_start(
                            xts[(c + 1) % 2][:],
                            xtv[:, bass.ds((c + 1) * NSUB, NSUB)])
                        nc.sync.dma_start(
                            xcs[(c + 1) % 2][:],
                            xv[:, :, bass.ds((c + 1) * CW, CW)])

                    # G[d, d'] += sum_n xT[n, d] xT[n, d'] for this chunk
                    # G is symmetric: compute only upper-tri spans
                    for ds_ in range(DT):
                        dsl = bass.ds(ds_ * P, P)
                        pieces = []
                        st = ds_ * P
                        while st < DX:
                            w = min(512, DX - st)
                            pieces.append((st, w))
                            st += w
                        g_ps = {st: ps.tile([P, w], f32, tag="ps",
                                            name=f"gps{c}_{ds_}_{st}")
                                for st, w in pieces}
                        for nt in range(NSUB):
                            for st, w in pieces:
                                nc.tensor.matmul(
                                    g_ps[st][:], xtc[:, nt, dsl],
                                    xtc[:, nt, bass.ds(st, w)],
                                    start=(nt == 0), stop=(nt == NSUB - 1))
                        for st, w in pieces:
                            dcs = bass.ds(st, w)
                            if c == 0:
                                nc.vector.tensor_copy(
                                    g_sb[ds_][:, dcs], g_ps[st][:])
                            else:
                                nc.vector.tensor_add(
                                    g_sb[ds_][:, dcs], g_sb[ds_][:, dcs],
                                    g_ps[st][:])

                    # V projection: psum [v-128, n-chunk]; on the last
                    # chunk it also covers the final G flush latency
                    for vt in range(KT):
                        vps = ps.tile([P, CW], f32, tag="ps",
                                      name=f"vps{c}_{vt}")
                        vsl = bass.ds(vt * P, P)
                        for dt in range(DT):
                            nc.tensor.matmul(
                                vps[:], wv_sb[dt][:, 0, vsl], xc[:, dt],
                                start=(dt == 0), stop=(dt == DT - 1))
                        nc.scalar.activation(
                            v_res[:, vt, bass.ds(c * CW, CW)], vps[:],
                            mybir.ActivationFunctionType.Copy)

            # ---- AT = G . Wk (per d'-sub): [d'-part, k] bf16 ----
            # (matmul cannot mix f32r with bf16 operands, so G rounds to
            # bf16 after accumulation; sim rel err 9.1e-3 vs 8.5e-3 f32r)
            with tc.tile_pool(name="pat", bufs=1) as pat:
                g_bf = [pat.tile([P, DX], bf16, tag=f"gb{dt}",
                                 name=f"gb{dt}") for dt in range(DT)]
                for dt in range(DT):
                    usl = bass.ds(dt * P, DX - dt * P)
                    nc.vector.tensor_copy(g_bf[dt][:, usl],
                                          g_sb[dt][:, usl])
                # fill all lower tiles via DMA XBAR transpose,
                # split across both HWDGE rings for 2x throughput
                ti = 0
                tq = [nc.sync, nc.scalar]
                for a in range(1, DT):
                    for b in range(a):
                        tq[ti % 2].dma_start(
                            g_bf[a][:, bass.ds(b * P, P)],
                            g_bf[b][:, bass.ds(a * P, P)],
                            transpose=True)
                        ti += 1
                at_sb = [pat.tile([P, DKH], bf16, tag=f"at{ds_}",
                                  name=f"at{ds_}") for ds_ in range(DT)]
                for ds_ in range(DT):
                    dsl = bass.ds(ds_ * P, P)
                    at_ps = ps.tile([P, DKH], f32, tag="ps",
                                    name=f"atps{ds_}")
                    for dt in range(DT):
                        nc.tensor.matmul(at_ps[:], g_bf[dt][:, dsl],
                                         wk_sb[dt][:, 0],
                                         start=(dt == 0),
                                         stop=(dt == DT - 1))
                    nc.vector.tensor_copy(at_sb[ds_][:], at_ps[:])

                # ---- scoresT[k, q] = AT.T Wq' + 1_k (x) vq ----
                for kt in range(KT):
                    ksl = bass.ds(kt * P, P)
                    for qc in range(QC):
                        qsl = bass.ds(qc * 512, 512)
                        s_ps = ps.tile([P, 512], f32, tag="ps",
                                       name=f"sps{kt}_{qc}")
                        for dt in range(DT):
                            nc.tensor.matmul(
                                s_ps[:], at_sb[dt][:, ksl],
                                wq_sb[dt][:, 0, qsl],
                                start=(dt == 0), stop=False)
                        # K mean-restore: rank-1, host-computed vq
                        nc.tensor.matmul(s_ps[:], ones_r[:],
                                         vq_r[:, qsl],
                                         start=False, stop=True)
                        nc.vector.tensor_copy(scores_sb[kt][:, qsl],
                                              s_ps[:])
                    softmax_kt(kt)

            # ---------------- softmax over q + out = pT.T @ V -------------
            with (
                tc.tile_pool(name="pout", bufs=4) as pout,
                tc.tile_pool(name="pseed", bufs=1) as pseed,
            ):
                seed_sb = pseed.tile([1, 1], f32, tag="seed")
                nc.sync.dma_start(seed_sb[:], seed.ap())
                outv = out.ap().rearrange("(qt p) n -> p qt n", p=P)
                NOC = N // OCW
                for c in range(NOC):
                    ncol = bass.ds(c * OCW, OCW)
                    if c == 0:
                        # kt-outer across all 8 psum banks: kt0 matmuls
                        # start as soon as p_r[0] lands, overlapping the
                        # softmax chain of kt1..3
                        ops = [ps.tile([P, OCW], f32, tag="ps",
                                       name=f"ops{c}_{qt}")
                               for qt in range(QT128)]
                        for kt in range(KT):
                            for qt in range(QT128):
                                nc.tensor.matmul(
                                    ops[qt][:],
                                    p_r[kt][:, bass.ds(qt * P, P)],
                                    v_res[:, kt, ncol],
                                    start=(kt == 0), stop=(kt == KT - 1))
                        for qg in range(QT128 // 4):
                            osb = pout.tile([P, 4, OCW], f32, tag="osb")
                            for qi in range(4):
                                qt = qg * 4 + qi
                                nc.vector.tensor_copy(osb[:, qi],
                                                      ops[qt][:])
                                if qt == 0:
                                    nc.vector.tensor_scalar_add(
                                        osb[0:1, 0, 0:1], ops[0][0:1, 0:1],
                                        seed_sb[:])
                                    if sink is not None:
                                        nc.sync.dma_start(
                                            sink.ap(), osb[0:1, 0, 0:1])
                            q = nc.gpsimd if qg % 2 else nc.sync
                            q.dma_start(
                                outv[:, qg * 4:(qg + 1) * 4, ncol], osb[:])
                    else:
                        # qt-major: evictions and stores trail each psum
                        # closely (fine-grained tail on the last chunk)
                        for qg in range(QT128 // 4):
                            osb = pout.tile([P, 4, OCW], f32, tag="osb")
                            for qi in range(4):
                                qt = qg * 4 + qi
                                ops = ps.tile([P, OCW], f32, tag="ps",
                                              name=f"ops{c}_{qt}")
                                for kt in range(KT):
                                    nc.tensor.matmul(
                                        ops[:],
                                        p_r[kt][:, bass.ds(qt * P, P)],
                                        v_res[:, kt, ncol],
                                        start=(kt == 0),
                                        stop=(kt == KT - 1))
                                if c == NOC - 1 and qt % 2:
                                    # alternate eviction engine on the
                                    # final chunk so the store tail is
                                    # paced by two engines, not one
                                    nc.scalar.activation(
                                        osb[:, qi], ops[:],
                                        mybir.ActivationFunctionType.Copy)
                                else:
                                    nc.vector.tensor_copy(osb[:, qi],
                                                          ops[:])
                                if c == NOC - 1:
                                    # store right behind each eviction,
                                    # alternating queues so the tail is
                                    # one 0.25MB transfer, not a backlog
                                    q = nc.scalar if qt % 2 else nc.gpsimd
                                    q.dma_start(
                                        outv[:, qt:qt + 1, ncol],
                                        osb[:, qi:qi + 1])
                            if c < NOC - 1:
                                # slab stores alternate rings: 16MB of
                                # output needs both to keep pace
                                q = nc.gpsimd if (c + qg) % 2 else nc.sync
                                q.dma_start(
                                    outv[:, qg * 4:(qg + 1) * 4, ncol],
                                    osb[:])

            if rep_cm is not None:
                rep_cm.__exit__(None, None, None)

    nc.compile()
    return nc


_CACHE = {}


def _get_nc(DX, N, DQ, DKH):
    key = (DX, N, DQ, DKH)
    if key not in _CACHE:
        _CACHE[key] = _build_core_kernel(DX, N, DQ, DKH)
    return _CACHE[key]


def _bf16(a):
    import ml_dtypes
    return np.ascontiguousarray(a.astype(ml_dtypes.bfloat16))


def _run(x, Wq, Wk, Wv, **spmd_kwargs):
    """Run the SPMD kernel; returns (out, BassKernelResults)."""
    from concourse.bass_utils import run_bass_kernel_spmd

    B, DX, N = x.shape
    DQ = Wq.shape[0]
    DK = Wk.shape[0]
    assert (B, DX, N, DQ, DK) == (B_FULL, DX_FULL, N_FULL, DQ_FULL, DK_FULL)
    DKH = DK // 2
    P = 128

    nc = _get_nc(DX, N, DQ, DKH)

    # Wq/Wk shipped mean-removed (entries - 0.5) in bf16. Q needs no
    # restore (cancels in softmax); K's restore is the host-exact
    # sh = 0.5*colsum(x), laid out [p, ntile].
    # 1/sqrt(DQ) folded into Wq: scores arrive pre-scaled for the softmax
    WqT = _bf16((np.ascontiguousarray(Wq.T, dtype=np.float32)
                 - np.float32(0.5)) * np.float32(1.0 / math.sqrt(DQ_FULL)))
    WkT = _bf16(np.ascontiguousarray(Wk.T, dtype=np.float32)
                - np.float32(0.5))
    WvT = _bf16(np.ascontiguousarray(Wv.T, dtype=np.float32))

    in_maps = []
    for c in range(N_CORES):
        b, h = divmod(c, 2)
        hsl = slice(h * DKH, (h + 1) * DKH)
        xbf = _bf16(x[b])
        # K mean-restore, rank-1 in q: v[q] = 0.5 * (Wq' (x colsum(x)))[q]
        # (constant across k -> added to scoresT as ones (x) vq on-chip)
        s = x[b].astype(np.float32).sum(axis=0)
        v = 0.5 * (WqT.astype(np.float32).T
                   @ (xbf.astype(np.float32) @ s))
        in_maps.append({
            "xb": xbf,
            "xt": _bf16(np.ascontiguousarray(x[b].T)),
            "wqt": WqT,
            "wkt": np.ascontiguousarray(WkT[:, hsl]),
            "wvt": np.ascontiguousarray(WvT[:, hsl]),
            "vq": np.ascontiguousarray(v[None, :].astype(np.float32)),
            "seed": np.zeros((1, 1), np.float32),
        })

    res = run_bass_kernel_spmd(nc, in_maps, core_ids=list(range(N_CORES)),
                               **spmd_kwargs)
    out = np.empty((B, DQ, N), np.float32)
    for b in range(B):
        out[b] = res.results[2 * b]["out"] + res.results[2 * b + 1]["out"]
    return out, res


def kernel(x, Wq, Wk, Wv):
    return _run(x, Wq, Wk, Wv)[0]

